# revision 51
# baseline (speedup 1.0000x reference)
"""Masked attention kernel for Trainium2, data-parallel over 8 NeuronCores.

Problem: out[q,b,:] = softmax-ish(LN(query Wq^T+bq) @ LN(key Wk^T+bk)^T / sqrt(H),
masked by query_mask & key_mask, with the reference's idiosyncratic
exp(s - 2*rowmax) / (sum + 0.001) normalization) @ value.

Key observations exploited:
 - The reference fills masked scores with the GLOBAL min before the row max.
   Every unmasked score >= global min, so the row max equals the max over
   unmasked entries whenever one exists; fully-masked rows output exactly 0.
   Hence zero cross-batch communication: B=8 batches map 1:1 onto 8 cores.
 - Masked-out query rows produce zero output rows; masked-out keys contribute
   nothing.  Both masks are ~50% dense, so each core computes attention only
   over compacted (host-gathered) rows, padded to a fixed size.
 - exp(s - 2m)/(sum + 0.001) == exp(s)/(sum' + 0.001*exp(2m)), and scaled
   scores are O(5) so exp needs no shift at all; the denominator is corrected
   by -npad (each padded key column contributes exactly exp(0)=1) and the
   +0.001 term (~3.5e-4 of the sum) is dropped.
 - All layout transposes (projection -> [h,seq] operands, exp(S) -> [k,q]
   stationaries for the PV matmul) run on the DMA engines' XBAR transpose
   path instead of the PE array, leaving the PE a pure matmul stream.

Engine/DMA choreography (v2):
 - The PE is the bottleneck (~50us of matmul streaming at bf16); everything
   else is scheduled around keeping it 100% fed:
   * a short burst of dummy warm-up matmuls runs while the first input DMAs
     are still in flight, so the HAM clock-gate un-throttles (1.2->2.4 GHz)
     before the first real matmul;
   * per-engine DMA issue cost is ~0.7-1us per dma_start, so the issue work
     is spread: sync carries the first x tiles then ONLY transposes (in
     dependency order, so its FIFO never head-of-line blocks), scalar
     carries the weight quarters, gpsimd (SWDGE) carries the bulk x/v
     stream interleaved between its per-job LN ops;
   * kT batches are 4 tiles = 512 columns = exactly one QK^T PSUM-bank
     group, so each score group's matmuls wait on exactly one transpose.
 - The exp activations emit the softmax denominator for free via accum_out,
   killing the vector reduce; the final context scale runs on the scalar
   engine (activation Copy w/ per-partition scale) where there is slack.
 - LN chain per projection job: vector bn_stats/bn_aggr -> scalar sqrt ->
   vector reciprocal -> gpsimd (-mean*rstd) -> scalar apply+cast (Identity
   with bias/scale), each engine loaded under the PE's 853ns/job budget.

Host side: compact/pad/transpose per batch (cheap numpy), run the SPMD NEFF,
scatter results back into the full [Q,B,H] output.
"""

import numpy as np
import ml_dtypes

import concourse.bacc as bacc
import concourse.bass as bass
import concourse.tile as tile
from concourse import mybir
from concourse.bass_utils import run_bass_kernel_spmd


def _ensure_axon_hooks():
    """concourse's trace path imports antenv.axon_hooks, which is absent in
    some containers; provide a no-op stand-in so BASS_TRACE=1 degrades to
    untraced execution instead of crashing."""
    try:
        import antenv.axon_hooks  # noqa: F401
    except ImportError:
        import sys as _sys
        import types as _types
        m = _types.ModuleType("antenv.axon_hooks")
        m._h = None
        m.set_axon_ntff_profile_hook = lambda h: setattr(m, "_h", h)
        m.get_axon_ntff_profile_hook = lambda: m._h
        _sys.modules["antenv.axon_hooks"] = m


_ensure_axon_hooks()

F32 = mybir.dt.float32
BF16 = mybir.dt.bfloat16
AX = mybir.AxisListType.X
AF = mybir.ActivationFunctionType
ALU = mybir.AluOpType

H = 512
HC = H // 128          # contraction chunks over the hidden dim
NCORES = 8
RSQRT_H = 1.0 / float(np.sqrt(np.float32(H)))
EPS = 1e-5
NWARM = 8              # dummy matmuls to lift the HAM clock gate

_cache = {}
last_results = None


def _build(pad, nkfree, biasq, biask, affq, affk):
    nt = pad // 128
    # QK^T PSUM-bank groups over the REAL key columns only (pad-key columns
    # beyond nkfree are never scored; their e is memset to 0)
    groups = [512] * (nkfree // 512)
    if nkfree % 512:
        groups.append(nkfree % 512)
    ng = len(groups)
    # input-tile DMA groups (granularity of arrival -> consumer wakeup):
    # K tiles 0-4 arrive as singles (they gate consecutive early jobs), the
    # tail as one batch; Q tiles in growing batches (need times spread out).
    xqbat = [1] * min(3, nt)
    while sum(xqbat) < nt:
        xqbat.append(min(3, nt - sum(xqbat)))
    xkbat = [1] * min(5, nt)
    while sum(xkbat) < nt:
        xkbat.append(min(2, nt - sum(xkbat)))

    def _bmap(bat):
        m = {}
        base = 0
        for i, b in enumerate(bat):
            for s2 in range(b):
                m[base + s2] = (i, s2, base, b)
            base += b
        return m
    xqmap, xkmap = _bmap(xqbat), _bmap(xkbat)

    nc = bacc.Bacc(None, target_bir_lowering=False, debug=False, enable_asserts=False,
                   enable_partition_id=False)

    xqT_d = nc.declare_dram_parameter("xqT", [nt, 128, HC, 128], BF16, isOutput=False)
    xkT_d = nc.declare_dram_parameter("xkT", [nt, 128, HC, 128], BF16, isOutput=False)
    v_d = nc.declare_dram_parameter("v", [pad, H], BF16, isOutput=False)
    npad_d = nc.declare_dram_parameter("npad", [1, 1], F32, isOutput=False)
    km_d = None
    if biask or affk:
        km_d = nc.declare_dram_parameter("km01", [pad, 1], F32, isOutput=False)
    wqT_d = nc.declare_dram_parameter("WqT", [H, H], BF16, isOutput=False)
    wkT_d = nc.declare_dram_parameter("WkT", [H, H], BF16, isOutput=False)
    extras_d = {}
    if biasq:
        extras_d["bq"] = nc.declare_dram_parameter("bq", [1, H], F32, isOutput=False)
    if biask:
        extras_d["bk"] = nc.declare_dram_parameter("bk", [1, H], F32, isOutput=False)
    if affq:
        extras_d["gq"] = nc.declare_dram_parameter("gq", [1, H], F32, isOutput=False)
        extras_d["betaq"] = nc.declare_dram_parameter("betaq", [1, H], F32, isOutput=False)
    if affk:
        extras_d["gk"] = nc.declare_dram_parameter("gk", [1, H], F32, isOutput=False)
        extras_d["betak"] = nc.declare_dram_parameter("betak", [1, H], F32, isOutput=False)
    out_d = nc.declare_dram_parameter("out", [pad, H], BF16, isOutput=True)

    with tile.TileContext(nc) as tc:
        with (
            tc.tile_pool(name="persist", bufs=1) as persist,
            tc.tile_pool(name="small", bufs=10) as small,
            tc.tile_pool(name="lnt", bufs=4) as lnt,
            tc.tile_pool(name="ework", bufs=7) as ework,
            tc.tile_pool(name="osb", bufs=3) as osbp,
            tc.tile_pool(name="ps", bufs=1, space="PSUM") as ps,
        ):
            eps_t = persist.tile([128, 1], F32)
            nc.vector.memset(eps_t[:], EPS)
            one_t = persist.tile([128, 1], F32)
            nc.vector.memset(one_t[:], 1.0)
            # warm-up operand: zeros so the dummy matmuls are numerically inert
            wz = persist.tile([128, H], BF16)
            nc.vector.memset(wz[:], 0.0)

            # ---- input DMA choreography --------------------------------------
            # Empirically the SWDGE (gpsimd) queue drains fastest and the
            # scalar HWDGE queue slowest, so: gpsimd carries the weights +
            # first tiles (everything phase A blocks on early), sync carries
            # the bulk x stream then ONLY transposes, scalar (busy computing)
            # carries just xq0 + v + npad.  All issues are unconditional and
            # sit at each engine's stream head.
            wq_h = [persist.tile([128, 2, H], BF16, tag=f"wqh{j}", name=f"wqh{j}")
                    for j in range(2)]
            wk_h = [persist.tile([128, 2, H], BF16, tag=f"wkh{j}", name=f"wkh{j}")
                    for j in range(2)]
            xqb = [persist.tile([128, xqbat[j], HC, 128], BF16, tag=f"xqb{j}", name=f"xqb{j}")
                   for j in range(len(xqbat))]
            xkb = [persist.tile([128, xkbat[j], HC, 128], BF16, tag=f"xkb{j}", name=f"xkb{j}")
                   for j in range(len(xkbat))]

            def _wslice(dram, h):
                return dram[256 * h:256 * (h + 1), :].rearrange("(c p) i -> p c i", p=128)

            def _xslice(dram, bat, j):
                t0 = sum(bat[:j]); t1 = t0 + bat[j]
                return dram[t0:t1, :, :, :].rearrange("t p c u -> p t c u")

            npad_t = persist.tile([128, 1], F32)
            kmA_sb = None
            if km_d is not None:
                kmA_sb = persist.tile([128, nt], F32)
            bcast = {}
            for name in extras_d:
                bcast[name] = persist.tile([128, H], F32, tag=f"bc_{name}", name=f"bc_{name}")

            # flat [128 h_sub, h_chunk, seq] operand layouts for attention:
            # QK^T's stationary (qT) and moving (kT) slices are arbitrary-
            # width contiguous column ranges, and each per-job transpose
            # lands as early as its LN completes.
            qT_f = persist.tile([128, HC, pad], BF16)
            kT_f = persist.tile([128, HC, pad], BF16)
            v_sb = persist.tile([128, nt, H], BF16)

            # Each HWDGE/SWDGE ring drains FIFO at ~1/3 of the ~320 GB/s
            # aggregate; pieces are placed so every ring's drain order matches
            # the projection jobs' consumption order (Q0, K0-8, Q1-8), no ring
            # carries more than ~2 pieces ahead of need, and the sync ring is
            # lean so the mid-kernel transposes aren't queued behind bulk.
            def _xq(j):
                return (xqb[j][:, 0:xqbat[j]], _xslice(xqT_d, xqbat, j))

            def _xk(j):
                return (xkb[j][:, 0:xkbat[j]], _xslice(xkT_d, xkbat, j))

            nxk, nxq = len(xkbat), len(xqbat)
            # scalar ring (slowest): only latest-need small pieces + v/npad
            o, i_ = _xq(0); nc.scalar.dma_start(out=o, in_=i_)
            o, i_ = _xk(0); nc.scalar.dma_start(out=o, in_=i_)
            if nxq > 3:
                o, i_ = _xq(3); nc.scalar.dma_start(out=o, in_=i_)
            nc.scalar.dma_start(out=v_sb[:],
                                in_=v_d[:, :].rearrange("(n p) h -> p n h", p=128))
            np_src = npad_d[:, :]
            np_src = bass.AP(tensor=np_src.tensor, offset=np_src.offset,
                             ap=[[0, 128], [1, 1]])
            nc.scalar.dma_start(out=npad_t[:], in_=np_src)
            # gpsimd ring (fastest): wq/wk first halves + odd xk pieces
            nc.gpsimd.dma_start(out=wq_h[0][:], in_=_wslice(wqT_d, 0))
            if nxq > 1:
                o, i_ = _xq(1); nc.gpsimd.dma_start(out=o, in_=i_)
            nc.gpsimd.dma_start(out=wk_h[0][:], in_=_wslice(wkT_d, 0))
            for j in range(1, nxk, 2):
                o, i_ = _xk(j); nc.gpsimd.dma_start(out=o, in_=i_)
            # sync ring: second halves + even xk + tail xq; lean thereafter
            # so the XBAR transposes aren't queued behind bulk
            nc.sync.dma_start(out=wq_h[1][:], in_=_wslice(wqT_d, 1))
            if nxq > 2:
                o, i_ = _xq(2); nc.sync.dma_start(out=o, in_=i_)
            nc.sync.dma_start(out=wk_h[1][:], in_=_wslice(wkT_d, 1))
            for j in range(2, nxk, 2):
                o, i_ = _xk(j); nc.sync.dma_start(out=o, in_=i_)
            for j in range(4, nxq):
                o, i_ = _xq(j); nc.sync.dma_start(out=o, in_=i_)
            if km_d is not None:
                nc.gpsimd.dma_start(out=kmA_sb[:],
                                    in_=km_d[:, :].rearrange("(n p) o -> p (n o)", p=128))
            for name, dram in extras_d.items():
                src = dram[:, :]
                src = bass.AP(tensor=src.tensor, offset=src.offset,
                              ap=[[0, 128]] + [src.ap[-1]])
                nc.gpsimd.dma_start(out=bcast[name][:], in_=src)



            # ---- PE warm-up: lift the HAM clock gate while DMAs stream ------
            for w in range(NWARM):
                pw = ps.tile([128, H], F32, tag="u", bufs=8, name=f"warm{w}")
                nc.tensor.matmul(pw[:], wz[:, 0:128], wz[:], start=True, stop=True)

            # ---- phase A: project + layernorm (XBAR-transposed evacuation) ---
            # First THREE Q jobs: their lnp batch completes at job 2, so the
            # qT0 transpose issues early and the K-side weights aren't needed
            # until job 3 (~2.6us later), easing the head DMA crunch.
            nq0 = min(3, nt)
            jobs = ([(0, t) for t in range(nq0)]
                    + [(1, t) for t in range(nt)]
                    + [(0, t) for t in range(nq0, nt)])
            ps_of = {}
            lnp_of = {}

            def proj(i):
                s, t = jobs[i]
                if s == 0:
                    xi, xs, _, _ = xqmap[t]
                    x_sb = xqb[xi][:, xs, :, :]
                else:
                    xi, xs, _, _ = xkmap[t]
                    x_sb = xkb[xi][:, xs, :, :]
                p = ps.tile([128, H], F32, tag="u", bufs=8)
                ps_of[i] = p
                w_h = (wq_h, wk_h)[s]
                for c in range(HC):
                    nc.tensor.matmul(p[:], x_sb[:, c, :], w_h[c // 2][:, c % 2, :],
                                     start=(c == 0), stop=(c == HC - 1))

            def apply_and_tp(i, p, rstd_ap):
                s, t = jobs[i]
                use_aff = (affq, affk)[s]
                ln = lnt.tile([128, H], BF16, tag="lnp")
                # projection outputs are zero-mean by construction (host
                # centers the weights), so LN-apply is a plain scale
                if use_aff:
                    ln32 = lnt.tile([128, H], F32, tag="ln32")
                    nc.scalar.activation(out=ln32[:], in_=p[:], func=AF.Identity,
                                         bias=0.0, scale=rstd_ap)
                    nc.vector.tensor_mul(ln32[:], ln32[:], bcast[("gq", "gk")[s]][:])
                    nc.vector.tensor_add(ln[:], ln32[:], bcast[("betaq", "betak")[s]][:])
                elif i % 14 == 13:
                    # occasional apply on the vector engine balances scalar
                    # (sqrt+apply ~930/job) against vector (stats+recip ~821)
                    nc.vector.tensor_scalar_mul(ln[:], p[:], rstd_ap)
                else:
                    nc.scalar.activation(out=ln[:], in_=p[:], func=AF.Identity,
                                         bias=0.0, scale=rstd_ap)
                if s == 1 and kmA_sb is not None:
                    nc.vector.tensor_scalar_mul(ln[:], ln[:], kmA_sb[:, t:t + 1])
                # per-job XBAR transpose into the flat [h_sub, chunk, seq]
                # operand: dst[p, c, t*128 + q] = ln[q, c*128 + p]
                dst = (qT_f, kT_f)[s][:, :, 128 * t:128 * (t + 1)]
                nc.sync.dma_start_transpose(out=dst, in_=ln[:])

            def ln_and_tp(i):
                s, t = jobs[i]
                use_bias = (biasq, biask)[s]
                p = ps_of.pop(i)
                if use_bias:
                    nc.vector.tensor_add(p[:], p[:], bcast[("bq", "bk")[s]][:])
                stats = small.tile([128, 6], F32, tag="stats")
                nc.vector.bn_stats(out=stats[:], in_=p[:])
                # zero-mean rows: var*H = nv_even + nv_odd + H*m_even^2; the
                # last term is ~0.4% of var (m_e ~ N(0, 2/H)) -> dropped, the
                # resulting ~0.2% per-row rstd jitter is far under tolerance
                v512 = small.tile([128, 1], F32, tag="v512")
                nc.gpsimd.tensor_tensor(out=v512[:], in0=stats[:, 2:3],
                                        in1=stats[:, 5:6], op=ALU.add)
                sd = small.tile([128, 1], F32, tag="sd")
                nc.scalar.activation(out=sd[:], in_=v512[:], func=AF.Sqrt,
                                     bias=eps_t[:], scale=1.0 / H)
                rstd = small.tile([128, 1], F32, tag="rstd")
                nc.vector.reciprocal(out=rstd[:], in_=sd[:])
                apply_and_tp(i, p, rstd[:])

            DEPTH = 7
            for i in range(min(DEPTH, len(jobs))):
                proj(i)
            for i in range(len(jobs)):
                if i + DEPTH < len(jobs):
                    proj(i + DEPTH)
                ln_and_tp(i)

            # ---- phase B: attention (SW-pipelined) ---------------------------
            eT_sb = persist.tile([128, nt, nt, 128], BF16)
            S_of = {}
            # scores/exps computed only over the first nkfree key columns
            # (the real keys); e[:, nkfree:pad] is zeroed so the transposed
            # tail rows contribute exactly nothing to PV
            goff = [sum(groups[:j]) for j in range(ng + 1)]

            def qk(t):
                Ss = [ps.tile([128, groups[j]], F32, tag="u", bufs=8, name=f"S{j}")
                      for j in range(ng)]
                S_of[t] = Ss
                # group-outer so each score bank finishes (and its exp can
                # start) while the next group's matmuls still stream
                for j in range(ng):
                    for c in range(HC):
                        nc.tensor.matmul(Ss[j][:], qT_f[:, c, 128 * t:128 * (t + 1)],
                                         kT_f[:, c, goff[j]:goff[j + 1]],
                                         start=(c == 0), stop=(c == HC - 1))

            def attend(t):
                Ss = S_of.pop(t)
                e = ework.tile([128, pad], BF16, tag="e")
                if nkfree < pad:
                    nc.vector.memset(e[:, nkfree:pad], 0.0)
                for j in range(ng):
                    nc.scalar.activation(out=e[:, goff[j]:goff[j + 1]], in_=Ss[j][:],
                                         func=AF.Exp, bias=0.0, scale=RSQRT_H)
                dsum = small.tile([128, 1], F32, tag="dsum")
                nc.vector.reduce_sum(dsum[:], e[:, 0:nkfree], axis=AX)
                # XBAR transpose: eT[p, kt, q] = e[q, kt*128 + p]
                nc.sync.dma_start_transpose(out=eT_sb[:, t, :, :], in_=e[:])
                # denom = sum(e) - npad  (each padded key contributes e=1;
                # the reference's +0.001 term is ~3.5e-4 of the sum: dropped)
                nc.gpsimd.tensor_scalar(out=dsum[:], in0=dsum[:], scalar1=npad_t[:],
                                        scalar2=None, op0=ALU.subtract)
                r = small.tile([128, 1], F32, tag="r")
                nc.vector.reciprocal(out=r[:], in_=dsum[:])

                C = ps.tile([128, H], F32, tag="u", bufs=8, name="C")
                for kt in range(nt):
                    nc.tensor.matmul(C[:], eT_sb[:, t, kt, :], v_sb[:, kt, :],
                                     start=(kt == 0), stop=(kt == nt - 1))
                o = osbp.tile([128, H], BF16, tag="o")
                nc.vector.tensor_scalar_mul(o[:], C[:], r[:])
                nc.gpsimd.dma_start(out=out_d[t * 128:(t + 1) * 128, :], in_=o[:])

            BDEPTH = 9
            for t in range(min(BDEPTH, nt)):
                qk(t)
            for t in range(nt):
                if t + BDEPTH < nt:
                    qk(t + BDEPTH)
                attend(t)

    nc.compile()
    return nc


def _get_nc(pad, nkfree, biasq, biask, affq, affk):
    key = (pad, nkfree, biasq, biask, affq, affk)
    if key not in _cache:
        _cache[key] = _build(*key)
    return _cache[key]


def kernel(query, key_in, value, query_mask, key_mask,
           Wq, bq, gq, betaq, Wk, bk, gk, betak):
    query = np.asarray(query, np.float32)
    key_in = np.asarray(key_in, np.float32)
    value = np.asarray(value, np.float32)
    query_mask = np.asarray(query_mask, bool)
    key_mask = np.asarray(key_mask, bool)
    Wq = np.asarray(Wq, np.float32); Wk = np.asarray(Wk, np.float32)
    bq = np.asarray(bq, np.float32); bk = np.asarray(bk, np.float32)
    gq = np.asarray(gq, np.float32); gk = np.asarray(gk, np.float32)
    betaq = np.asarray(betaq, np.float32); betak = np.asarray(betak, np.float32)

    Q, B, Hh = query.shape
    assert Hh == H and B == NCORES

    qidx = [np.nonzero(query_mask[:, b])[0] for b in range(B)]
    kidx = [np.nonzero(key_mask[:, b])[0] for b in range(B)]
    maxn = max([len(i) for i in qidx + kidx] + [1])
    pad = max(1152, -(-maxn // 128) * 128)

    biasq = bool(np.any(bq)); biask = bool(np.any(bk))
    affq = not (np.all(gq == 1.0) and not np.any(betaq))
    affk = not (np.all(gk == 1.0) and not np.any(betak))
    # number of key columns actually scored (max real keys across cores)
    nkfree = min(pad, max(max(len(i) for i in kidx), 1))
    nc = _get_nc(pad, nkfree, biasq, biask, affq, affk)

    # center the projections on the host: y = x @ (W - mean_rows(W))^T (+ b -
    # mean(b)) has exactly zero row-mean, so the kernel's LN skips the mean
    # entirely (its bn_stats variance formula relies on this)
    Wqc = Wq - Wq.mean(axis=0, keepdims=True)
    Wkc = Wk - Wk.mean(axis=0, keepdims=True)
    bq = bq - bq.mean(); bk = bk - bk.mean()
    wqT = np.ascontiguousarray(Wqc.T).astype(ml_dtypes.bfloat16)
    wkT = np.ascontiguousarray(Wkc.T).astype(ml_dtypes.bfloat16)
    in_maps = []
    for b in range(B):
        qi, ki = qidx[b], kidx[b]
        xq = np.zeros((pad, H), ml_dtypes.bfloat16)
        xq[:len(qi)] = query[qi, b].astype(ml_dtypes.bfloat16)
        xk = np.zeros((pad, H), ml_dtypes.bfloat16)
        xk[:len(ki)] = key_in[ki, b].astype(ml_dtypes.bfloat16)
        vv = np.zeros((pad, H), ml_dtypes.bfloat16)
        vv[:len(ki)] = value[ki, b].astype(ml_dtypes.bfloat16)
        nt = pad // 128
        # tile-major layout [nt, 128(p), HC, 128(u)]: per-tile DMA reads are
        # 1KB-contiguous per partition
        xqt = np.ascontiguousarray(xq.reshape(nt, 128, H // 128, 128).transpose(0, 3, 2, 1))
        xkt = np.ascontiguousarray(xk.reshape(nt, 128, H // 128, 128).transpose(0, 3, 2, 1))
        m = {
            "xqT": xqt,
            "xkT": xkt,
            "v": vv,
            "npad": np.full((1, 1), nkfree - len(ki), np.float32),
            "WqT": wqT,
            "WkT": wkT,
        }
        if biask or affk:
            km01 = np.zeros((pad, 1), np.float32); km01[:len(ki)] = 1.0
            m["km01"] = km01
        if biasq: m["bq"] = bq.reshape(1, H)
        if biask: m["bk"] = bk.reshape(1, H)
        if affq: m["gq"] = gq.reshape(1, H); m["betaq"] = betaq.reshape(1, H)
        if affk: m["gk"] = gk.reshape(1, H); m["betak"] = betak.reshape(1, H)
        in_maps.append(m)

    res = run_bass_kernel_spmd(nc, in_maps, core_ids=list(range(NCORES)))
    global last_results
    last_results = res

    out = np.zeros((Q, B, H), np.float32)
    for b in range(B):
        qi = qidx[b]
        out[qi, b, :] = res.results[b]["out"][:len(qi)]
    return out


# revision 60
# speedup vs baseline: 1.1281x; 1.1281x over previous
"""Masked attention kernel for Trainium2, data-parallel over 8 NeuronCores.

Problem: out[q,b,:] = softmax-ish(LN(query Wq^T+bq) @ LN(key Wk^T+bk)^T / sqrt(H),
masked by query_mask & key_mask, with the reference's idiosyncratic
exp(s - 2*rowmax) / (sum + 0.001) normalization) @ value.

Key observations exploited:
 - The reference fills masked scores with the GLOBAL min before the row max.
   Every unmasked score >= global min, so the row max equals the max over
   unmasked entries whenever one exists; fully-masked rows output exactly 0.
   Hence zero cross-batch communication: B=8 batches map 1:1 onto 8 cores.
 - Masked-out query rows produce zero output rows; masked-out keys contribute
   nothing.  Both masks are ~50% dense, so each core computes attention only
   over compacted (host-gathered) rows, padded to a fixed size.
 - exp(s - 2m)/(sum + 0.001) == exp(s)/(sum' + 0.001*exp(2m)), and scaled
   scores are O(5) so exp needs no shift at all; the denominator is corrected
   by -npad (each padded key column contributes exactly exp(0)=1) and the
   +0.001 term (~3.5e-4 of the sum) is dropped.
 - All layout transposes (projection -> [h,seq] operands, exp(S) -> [k,q]
   stationaries for the PV matmul) run on the DMA engines' XBAR transpose
   path instead of the PE array, leaving the PE a pure matmul stream.

Engine/DMA choreography (v2):
 - The PE is the bottleneck (~50us of matmul streaming at bf16); everything
   else is scheduled around keeping it 100% fed:
   * a short burst of dummy warm-up matmuls runs while the first input DMAs
     are still in flight, so the HAM clock-gate un-throttles (1.2->2.4 GHz)
     before the first real matmul;
   * per-engine DMA issue cost is ~0.7-1us per dma_start, so the issue work
     is spread: sync carries the first x tiles then ONLY transposes (in
     dependency order, so its FIFO never head-of-line blocks), scalar
     carries the weight quarters, gpsimd (SWDGE) carries the bulk x/v
     stream interleaved between its per-job LN ops;
   * kT batches are 4 tiles = 512 columns = exactly one QK^T PSUM-bank
     group, so each score group's matmuls wait on exactly one transpose.
 - The exp activations emit the softmax denominator for free via accum_out,
   killing the vector reduce; the final context scale runs on the scalar
   engine (activation Copy w/ per-partition scale) where there is slack.
 - LN chain per projection job: vector bn_stats/bn_aggr -> scalar sqrt ->
   vector reciprocal -> gpsimd (-mean*rstd) -> scalar apply+cast (Identity
   with bias/scale), each engine loaded under the PE's 853ns/job budget.

Host side: compact/pad/transpose per batch (cheap numpy), run the SPMD NEFF,
scatter results back into the full [Q,B,H] output.
"""

import numpy as np
import ml_dtypes

import concourse.bacc as bacc
import concourse.bass as bass
import concourse.tile as tile
from concourse import mybir
from concourse.bass_utils import run_bass_kernel_spmd


def _ensure_axon_hooks():
    """concourse's trace path imports antenv.axon_hooks, which is absent in
    some containers; provide a no-op stand-in so BASS_TRACE=1 degrades to
    untraced execution instead of crashing."""
    try:
        import antenv.axon_hooks  # noqa: F401
    except ImportError:
        import sys as _sys
        import types as _types
        m = _types.ModuleType("antenv.axon_hooks")
        m._h = None
        m.set_axon_ntff_profile_hook = lambda h: setattr(m, "_h", h)
        m.get_axon_ntff_profile_hook = lambda: m._h
        _sys.modules["antenv.axon_hooks"] = m


_ensure_axon_hooks()

F32 = mybir.dt.float32
BF16 = mybir.dt.bfloat16
AX = mybir.AxisListType.X
AF = mybir.ActivationFunctionType
ALU = mybir.AluOpType

H = 512
HC = H // 128          # contraction chunks over the hidden dim
NCORES = 8
RSQRT_H = 1.0 / float(np.sqrt(np.float32(H)))
EPS = 1e-5
NWARM = 8              # dummy matmuls to lift the HAM clock gate

_cache = {}
last_results = None


def _build(pad, nkfree, biasq, biask, affq, affk):
    nt = pad // 128
    # kT transpose batches double as the QK^T PSUM-bank groups (so each
    # group's matmuls depend on exactly one transpose's tile), sized so each
    # batch's issue+transfer completes before its QK consumers.
    kbat = [min(4, nt)]
    if nt > 4:
        kbat.append(min(3, nt - 4))
    while sum(kbat) < nt:
        kbat.append(min(2, nt - sum(kbat)))
    groups = list(kbat)
    ng = len(groups)
    qbat = [min(3, nt)]
    while sum(qbat) < nt:
        qbat.append(min(3, nt - sum(qbat)))
    # input-tile DMA groups (granularity of arrival -> consumer wakeup):
    # K tiles 0-4 arrive as singles (they gate consecutive early jobs), the
    # tail as one batch; Q tiles in growing batches (need times spread out).
    xqbat = [1] * min(3, nt)
    while sum(xqbat) < nt:
        xqbat.append(min(3, nt - sum(xqbat)))
    xkbat = [1] * min(5, nt)
    while sum(xkbat) < nt:
        xkbat.append(min(2, nt - sum(xkbat)))

    def _bmap(bat):
        m = {}
        base = 0
        for i, b in enumerate(bat):
            for s2 in range(b):
                m[base + s2] = (i, s2, base, b)
            base += b
        return m
    xqmap, xkmap = _bmap(xqbat), _bmap(xkbat)
    qmap, kmap = _bmap(qbat), _bmap(kbat)

    nc = bacc.Bacc(None, target_bir_lowering=False, debug=False, enable_asserts=False,
                   enable_partition_id=False)

    xqT_d = nc.declare_dram_parameter("xqT", [nt, 128, HC, 128], BF16, isOutput=False)
    xkT_d = nc.declare_dram_parameter("xkT", [nt, 128, HC, 128], BF16, isOutput=False)
    v_d = nc.declare_dram_parameter("v", [pad, H], BF16, isOutput=False)
    npad_d = nc.declare_dram_parameter("npad", [1, 1], F32, isOutput=False)
    km_d = None
    if biask or affk:
        km_d = nc.declare_dram_parameter("km01", [pad, 1], F32, isOutput=False)
    wqT_d = nc.declare_dram_parameter("WqT", [H, H], BF16, isOutput=False)
    wkT_d = nc.declare_dram_parameter("WkT", [H, H], BF16, isOutput=False)
    extras_d = {}
    if biasq:
        extras_d["bq"] = nc.declare_dram_parameter("bq", [1, H], F32, isOutput=False)
    if biask:
        extras_d["bk"] = nc.declare_dram_parameter("bk", [1, H], F32, isOutput=False)
    if affq:
        extras_d["gq"] = nc.declare_dram_parameter("gq", [1, H], F32, isOutput=False)
        extras_d["betaq"] = nc.declare_dram_parameter("betaq", [1, H], F32, isOutput=False)
    if affk:
        extras_d["gk"] = nc.declare_dram_parameter("gk", [1, H], F32, isOutput=False)
        extras_d["betak"] = nc.declare_dram_parameter("betak", [1, H], F32, isOutput=False)
    out_d = nc.declare_dram_parameter("out", [pad, H], BF16, isOutput=True)

    with tile.TileContext(nc) as tc:
        with (
            tc.tile_pool(name="persist", bufs=1) as persist,
            tc.tile_pool(name="small", bufs=10) as small,
            tc.tile_pool(name="lnt", bufs=4) as lnt,
            tc.tile_pool(name="ework", bufs=7) as ework,
            tc.tile_pool(name="osb", bufs=3) as osbp,
            tc.tile_pool(name="ps", bufs=1, space="PSUM") as ps,
        ):
            eps_t = persist.tile([128, 1], F32)
            nc.vector.memset(eps_t[:], EPS)
            one_t = persist.tile([128, 1], F32)
            nc.vector.memset(one_t[:], 1.0)
            # warm-up operand: zeros so the dummy matmuls are numerically inert
            wz = persist.tile([128, H], BF16)
            nc.vector.memset(wz[:], 0.0)

            # ---- input DMA choreography --------------------------------------
            # Empirically the SWDGE (gpsimd) queue drains fastest and the
            # scalar HWDGE queue slowest, so: gpsimd carries the weights +
            # first tiles (everything phase A blocks on early), sync carries
            # the bulk x stream then ONLY transposes, scalar (busy computing)
            # carries just xq0 + v + npad.  All issues are unconditional and
            # sit at each engine's stream head.
            wq_h = [persist.tile([128, 2, H], BF16, tag=f"wqh{j}", name=f"wqh{j}")
                    for j in range(2)]
            wk_h = [persist.tile([128, 2, H], BF16, tag=f"wkh{j}", name=f"wkh{j}")
                    for j in range(2)]
            xqb = [persist.tile([128, xqbat[j], HC, 128], BF16, tag=f"xqb{j}", name=f"xqb{j}")
                   for j in range(len(xqbat))]
            xkb = [persist.tile([128, xkbat[j], HC, 128], BF16, tag=f"xkb{j}", name=f"xkb{j}")
                   for j in range(len(xkbat))]

            def _wslice(dram, h):
                return dram[256 * h:256 * (h + 1), :].rearrange("(c p) i -> p c i", p=128)

            def _xslice(dram, bat, j):
                t0 = sum(bat[:j]); t1 = t0 + bat[j]
                return dram[t0:t1, :, :, :].rearrange("t p c u -> p t c u")

            npad_t = persist.tile([128, 1], F32)
            kmA_sb = None
            if km_d is not None:
                kmA_sb = persist.tile([128, nt], F32)
            bcast = {}
            for name in extras_d:
                bcast[name] = persist.tile([128, H], F32, tag=f"bc_{name}", name=f"bc_{name}")

            # [128 h_sub, tile, h_chunk, 128 seq] operand layouts for
            # attention.  kT is one tile PER transpose batch (= PSUM group)
            # so each group's matmuls depend on exactly one DMA; qT likewise.
            qTb = [persist.tile([128, qbat[j], HC, 128], BF16, tag=f"qTb{j}", name=f"qTb{j}")
                   for j in range(len(qbat))]
            kTg = [persist.tile([128, kbat[j], HC, 128], BF16, tag=f"kTg{j}", name=f"kTg{j}")
                   for j in range(len(kbat))]
            lnpK = [persist.tile([128, kbat[j], H], BF16, tag=f"lnpK{j}", name=f"lnpK{j}")
                    for j in range(len(kbat))]
            lnpQ = [persist.tile([128, qbat[j], H], BF16, tag=f"lnpQ{j}", name=f"lnpQ{j}")
                    for j in range(len(qbat))]
            v_sb = persist.tile([128, nt, H], BF16)

            # Each HWDGE/SWDGE ring drains FIFO at ~1/3 of the ~320 GB/s
            # aggregate; pieces are placed so every ring's drain order matches
            # the projection jobs' consumption order (Q0, K0-8, Q1-8), no ring
            # carries more than ~2 pieces ahead of need, and the sync ring is
            # lean so the mid-kernel transposes aren't queued behind bulk.
            def _xq(j):
                return (xqb[j][:, 0:xqbat[j]], _xslice(xqT_d, xqbat, j))

            def _xk(j):
                return (xkb[j][:, 0:xkbat[j]], _xslice(xkT_d, xkbat, j))

            nxk, nxq = len(xkbat), len(xqbat)
            # scalar ring (slowest): only latest-need small pieces + v/npad
            o, i_ = _xq(0); nc.scalar.dma_start(out=o, in_=i_)
            o, i_ = _xk(0); nc.scalar.dma_start(out=o, in_=i_)
            if nxq > 3:
                o, i_ = _xq(3); nc.scalar.dma_start(out=o, in_=i_)
            nc.scalar.dma_start(out=v_sb[:],
                                in_=v_d[:, :].rearrange("(n p) h -> p n h", p=128))
            np_src = npad_d[:, :]
            np_src = bass.AP(tensor=np_src.tensor, offset=np_src.offset,
                             ap=[[0, 128], [1, 1]])
            nc.scalar.dma_start(out=npad_t[:], in_=np_src)
            # gpsimd ring (fastest): wq/wk first halves + odd xk pieces
            nc.gpsimd.dma_start(out=wq_h[0][:], in_=_wslice(wqT_d, 0))
            if nxq > 1:
                o, i_ = _xq(1); nc.gpsimd.dma_start(out=o, in_=i_)
            nc.gpsimd.dma_start(out=wk_h[0][:], in_=_wslice(wkT_d, 0))
            for j in range(1, nxk, 2):
                o, i_ = _xk(j); nc.gpsimd.dma_start(out=o, in_=i_)
            # sync ring: second halves + even xk + tail xq; lean thereafter
            # so the XBAR transposes aren't queued behind bulk
            nc.sync.dma_start(out=wq_h[1][:], in_=_wslice(wqT_d, 1))
            if nxq > 2:
                o, i_ = _xq(2); nc.sync.dma_start(out=o, in_=i_)
            nc.sync.dma_start(out=wk_h[1][:], in_=_wslice(wkT_d, 1))
            for j in range(2, nxk, 2):
                o, i_ = _xk(j); nc.sync.dma_start(out=o, in_=i_)
            for j in range(4, nxq):
                o, i_ = _xq(j); nc.sync.dma_start(out=o, in_=i_)
            if km_d is not None:
                nc.gpsimd.dma_start(out=kmA_sb[:],
                                    in_=km_d[:, :].rearrange("(n p) o -> p (n o)", p=128))
            for name, dram in extras_d.items():
                src = dram[:, :]
                src = bass.AP(tensor=src.tensor, offset=src.offset,
                              ap=[[0, 128]] + [src.ap[-1]])
                nc.gpsimd.dma_start(out=bcast[name][:], in_=src)



            # ---- PE warm-up: lift the HAM clock gate while DMAs stream ------
            for w in range(NWARM):
                pw = ps.tile([128, H], F32, tag="u", bufs=8, name=f"warm{w}")
                nc.tensor.matmul(pw[:], wz[:, 0:128], wz[:], start=True, stop=True)

            # ---- phase A: project + layernorm (XBAR-transposed evacuation) ---
            # First THREE Q jobs: their lnp batch completes at job 2, so the
            # qT0 transpose issues early and the K-side weights aren't needed
            # until job 3 (~2.6us later), easing the head DMA crunch.
            nq0 = min(3, nt)
            jobs = ([(0, t) for t in range(nq0)]
                    + [(1, t) for t in range(nt)]
                    + [(0, t) for t in range(nq0, nt)])
            ps_of = {}
            lnp_of = {}

            def proj(i):
                s, t = jobs[i]
                if s == 0:
                    xi, xs, _, _ = xqmap[t]
                    x_sb = xqb[xi][:, xs, :, :]
                else:
                    xi, xs, _, _ = xkmap[t]
                    x_sb = xkb[xi][:, xs, :, :]
                p = ps.tile([128, H], F32, tag="u", bufs=8)
                ps_of[i] = p
                w_h = (wq_h, wk_h)[s]
                for c in range(HC):
                    nc.tensor.matmul(p[:], x_sb[:, c, :], w_h[c // 2][:, c % 2, :],
                                     start=(c == 0), stop=(c == HC - 1))

            def apply_and_tp(i, p, rstd_ap):
                s, t = jobs[i]
                use_aff = (affq, affk)[s]
                pid, slot, base, bsz = (qmap, kmap)[s][t]
                lnp = (lnpQ, lnpK)[s][pid]
                ln = lnp[:, slot, :]
                # projection outputs are zero-mean by construction (host
                # centers the weights), so LN-apply is a plain scale
                if use_aff:
                    ln32 = lnt.tile([128, H], F32, tag="ln32")
                    nc.scalar.activation(out=ln32[:], in_=p[:], func=AF.Identity,
                                         bias=0.0, scale=rstd_ap)
                    nc.vector.tensor_mul(ln32[:], ln32[:], bcast[("gq", "gk")[s]][:])
                    nc.vector.tensor_add(ln, ln32[:], bcast[("betaq", "betak")[s]][:])
                elif i % 14 == 13:
                    # occasional apply on the vector engine balances scalar
                    # (sqrt+apply ~930/job) against vector (stats+recip ~821)
                    nc.vector.tensor_scalar_mul(ln, p[:], rstd_ap)
                else:
                    nc.scalar.activation(out=ln, in_=p[:], func=AF.Identity,
                                         bias=0.0, scale=rstd_ap)
                if s == 1 and kmA_sb is not None:
                    nc.vector.tensor_scalar_mul(ln, ln, kmA_sb[:, t:t + 1])
                # batched XBAR transpose on sync: per 128-col block b,
                # dst[p, b, c, q] = ln[q, b, c*128+p]
                if slot == bsz - 1:
                    dst = (qTb, kTg)[s][pid][:, 0:bsz, :, :]
                    nc.sync.dma_start_transpose(out=dst, in_=lnp[:, 0:bsz, :])

            def ln_and_tp(i):
                s, t = jobs[i]
                use_bias = (biasq, biask)[s]
                p = ps_of.pop(i)
                if use_bias:
                    nc.vector.tensor_add(p[:], p[:], bcast[("bq", "bk")[s]][:])
                stats = small.tile([128, 6], F32, tag="stats")
                nc.vector.bn_stats(out=stats[:], in_=p[:])
                # zero-mean rows: var*H = nv_even + nv_odd + H*m_even^2; the
                # last term is ~0.4% of var (m_e ~ N(0, 2/H)) -> dropped, the
                # resulting ~0.2% per-row rstd jitter is far under tolerance
                v512 = small.tile([128, 1], F32, tag="v512")
                nc.gpsimd.tensor_tensor(out=v512[:], in0=stats[:, 2:3],
                                        in1=stats[:, 5:6], op=ALU.add)
                sd = small.tile([128, 1], F32, tag="sd")
                nc.scalar.activation(out=sd[:], in_=v512[:], func=AF.Sqrt,
                                     bias=eps_t[:], scale=1.0 / H)
                rstd = small.tile([128, 1], F32, tag="rstd")
                nc.vector.reciprocal(out=rstd[:], in_=sd[:])
                apply_and_tp(i, p, rstd[:])

            DEPTH = 7
            for i in range(min(DEPTH, len(jobs))):
                proj(i)
            for i in range(len(jobs)):
                if i + DEPTH < len(jobs):
                    proj(i + DEPTH)
                ln_and_tp(i)

            # ---- phase B: attention (SW-pipelined) ---------------------------
            eT_sb = persist.tile([128, nt, nt, 128], BF16)
            S_of = {}
            goff = [128 * sum(groups[:j]) for j in range(ng + 1)]

            def qk(t):
                Ss = [ps.tile([128, 128 * groups[j]], F32, tag="u", bufs=8, name=f"S{j}")
                      for j in range(ng)]
                S_of[t] = Ss
                qi, qs, _, _ = qmap[t]
                # group-outer so each score bank finishes (and its exp can
                # start) while the next group's matmuls still stream
                for j in range(ng):
                    for c in range(HC):
                        nc.tensor.matmul(Ss[j][:], qTb[qi][:, qs, c, :],
                                         kTg[j][:, :, c, :],
                                         start=(c == 0), stop=(c == HC - 1))

            def attend(t):
                Ss = S_of.pop(t)
                e = ework.tile([128, pad], BF16, tag="e")
                for j in range(ng):
                    nc.scalar.activation(out=e[:, goff[j]:goff[j + 1]], in_=Ss[j][:],
                                         func=AF.Exp, bias=0.0, scale=RSQRT_H)
                dsum = small.tile([128, 1], F32, tag="dsum")
                nc.vector.reduce_sum(dsum[:], e[:], axis=AX)
                # XBAR transpose: eT[p, kt, q] = e[q, kt*128 + p]
                nc.sync.dma_start_transpose(out=eT_sb[:, t, :, :], in_=e[:])
                # denom = sum(e) - npad  (each padded key contributes e=1;
                # the reference's +0.001 term is ~3.5e-4 of the sum: dropped)
                nc.gpsimd.tensor_scalar(out=dsum[:], in0=dsum[:], scalar1=npad_t[:],
                                        scalar2=None, op0=ALU.subtract)
                r = small.tile([128, 1], F32, tag="r")
                nc.vector.reciprocal(out=r[:], in_=dsum[:])

                C = ps.tile([128, H], F32, tag="u", bufs=8, name="C")
                for kt in range(nt):
                    nc.tensor.matmul(C[:], eT_sb[:, t, kt, :], v_sb[:, kt, :],
                                     start=(kt == 0), stop=(kt == nt - 1))
                o = osbp.tile([128, H], BF16, tag="o")
                nc.vector.tensor_scalar_mul(o[:], C[:], r[:])
                nc.gpsimd.dma_start(out=out_d[t * 128:(t + 1) * 128, :], in_=o[:])

            BDEPTH = 9
            for t in range(min(BDEPTH, nt)):
                qk(t)
            for t in range(nt):
                if t + BDEPTH < nt:
                    qk(t + BDEPTH)
                attend(t)

    nc.compile()
    return nc


def _get_nc(pad, nkfree, biasq, biask, affq, affk):
    key = (pad, nkfree, biasq, biask, affq, affk)
    if key not in _cache:
        _cache[key] = _build(*key)
    return _cache[key]


def kernel(query, key_in, value, query_mask, key_mask,
           Wq, bq, gq, betaq, Wk, bk, gk, betak):
    query = np.asarray(query, np.float32)
    key_in = np.asarray(key_in, np.float32)
    value = np.asarray(value, np.float32)
    query_mask = np.asarray(query_mask, bool)
    key_mask = np.asarray(key_mask, bool)
    Wq = np.asarray(Wq, np.float32); Wk = np.asarray(Wk, np.float32)
    bq = np.asarray(bq, np.float32); bk = np.asarray(bk, np.float32)
    gq = np.asarray(gq, np.float32); gk = np.asarray(gk, np.float32)
    betaq = np.asarray(betaq, np.float32); betak = np.asarray(betak, np.float32)

    Q, B, Hh = query.shape
    assert Hh == H and B == NCORES

    qidx = [np.nonzero(query_mask[:, b])[0] for b in range(B)]
    kidx = [np.nonzero(key_mask[:, b])[0] for b in range(B)]
    maxn = max([len(i) for i in qidx + kidx] + [1])
    pad = max(1152, -(-maxn // 128) * 128)

    biasq = bool(np.any(bq)); biask = bool(np.any(bk))
    affq = not (np.all(gq == 1.0) and not np.any(betaq))
    affk = not (np.all(gk == 1.0) and not np.any(betak))
    # number of key columns actually scored (max real keys across cores)
    nkfree = min(pad, max(max(len(i) for i in kidx), 1))
    nc = _get_nc(pad, nkfree, biasq, biask, affq, affk)

    # center the projections on the host: y = x @ (W - mean_rows(W))^T (+ b -
    # mean(b)) has exactly zero row-mean, so the kernel's LN skips the mean
    # entirely (its bn_stats variance formula relies on this)
    Wqc = Wq - Wq.mean(axis=0, keepdims=True)
    Wkc = Wk - Wk.mean(axis=0, keepdims=True)
    bq = bq - bq.mean(); bk = bk - bk.mean()
    wqT = np.ascontiguousarray(Wqc.T).astype(ml_dtypes.bfloat16)
    wkT = np.ascontiguousarray(Wkc.T).astype(ml_dtypes.bfloat16)
    in_maps = []
    for b in range(B):
        qi, ki = qidx[b], kidx[b]
        xq = np.zeros((pad, H), ml_dtypes.bfloat16)
        xq[:len(qi)] = query[qi, b].astype(ml_dtypes.bfloat16)
        xk = np.zeros((pad, H), ml_dtypes.bfloat16)
        xk[:len(ki)] = key_in[ki, b].astype(ml_dtypes.bfloat16)
        vv = np.zeros((pad, H), ml_dtypes.bfloat16)
        vv[:len(ki)] = value[ki, b].astype(ml_dtypes.bfloat16)
        nt = pad // 128
        # tile-major layout [nt, 128(p), HC, 128(u)]: per-tile DMA reads are
        # 1KB-contiguous per partition
        xqt = np.ascontiguousarray(xq.reshape(nt, 128, H // 128, 128).transpose(0, 3, 2, 1))
        xkt = np.ascontiguousarray(xk.reshape(nt, 128, H // 128, 128).transpose(0, 3, 2, 1))
        m = {
            "xqT": xqt,
            "xkT": xkt,
            "v": vv,
            "npad": np.full((1, 1), pad - len(ki), np.float32),
            "WqT": wqT,
            "WkT": wkT,
        }
        if biask or affk:
            km01 = np.zeros((pad, 1), np.float32); km01[:len(ki)] = 1.0
            m["km01"] = km01
        if biasq: m["bq"] = bq.reshape(1, H)
        if biask: m["bk"] = bk.reshape(1, H)
        if affq: m["gq"] = gq.reshape(1, H); m["betaq"] = betaq.reshape(1, H)
        if affk: m["gk"] = gk.reshape(1, H); m["betak"] = betak.reshape(1, H)
        in_maps.append(m)

    res = run_bass_kernel_spmd(nc, in_maps, core_ids=list(range(NCORES)))
    global last_results
    last_results = res

    out = np.zeros((Q, B, H), np.float32)
    for b in range(B):
        qi = qidx[b]
        out[qi, b, :] = res.results[b]["out"][:len(qi)]
    return out


# revision 62
# speedup vs baseline: 1.1447x; 1.0147x over previous
"""Masked attention kernel for Trainium2, data-parallel over 8 NeuronCores.

Problem: out[q,b,:] = softmax-ish(LN(query Wq^T+bq) @ LN(key Wk^T+bk)^T / sqrt(H),
masked by query_mask & key_mask, with the reference's idiosyncratic
exp(s - 2*rowmax) / (sum + 0.001) normalization) @ value.

Key observations exploited:
 - The reference fills masked scores with the GLOBAL min before the row max.
   Every unmasked score >= global min, so the row max equals the max over
   unmasked entries whenever one exists; fully-masked rows output exactly 0.
   Hence zero cross-batch communication: B=8 batches map 1:1 onto 8 cores.
 - Masked-out query rows produce zero output rows; masked-out keys contribute
   nothing.  Both masks are ~50% dense, so each core computes attention only
   over compacted (host-gathered) rows, padded to a fixed size.
 - exp(s - 2m)/(sum + 0.001) == exp(s)/(sum' + 0.001*exp(2m)), and scaled
   scores are O(5) so exp needs no shift at all; the denominator is corrected
   by -npad (each padded key column contributes exactly exp(0)=1) and the
   +0.001 term (~3.5e-4 of the sum) is dropped.
 - All layout transposes (projection -> [h,seq] operands, exp(S) -> [k,q]
   stationaries for the PV matmul) run on the DMA engines' XBAR transpose
   path instead of the PE array, leaving the PE a pure matmul stream.

Engine/DMA choreography (v2):
 - The PE is the bottleneck (~50us of matmul streaming at bf16); everything
   else is scheduled around keeping it 100% fed:
   * a short burst of dummy warm-up matmuls runs while the first input DMAs
     are still in flight, so the HAM clock-gate un-throttles (1.2->2.4 GHz)
     before the first real matmul;
   * per-engine DMA issue cost is ~0.7-1us per dma_start, so the issue work
     is spread: sync carries the first x tiles then ONLY transposes (in
     dependency order, so its FIFO never head-of-line blocks), scalar
     carries the weight quarters, gpsimd (SWDGE) carries the bulk x/v
     stream interleaved between its per-job LN ops;
   * kT batches are 4 tiles = 512 columns = exactly one QK^T PSUM-bank
     group, so each score group's matmuls wait on exactly one transpose.
 - The exp activations emit the softmax denominator for free via accum_out,
   killing the vector reduce; the final context scale runs on the scalar
   engine (activation Copy w/ per-partition scale) where there is slack.
 - LN chain per projection job: vector bn_stats/bn_aggr -> scalar sqrt ->
   vector reciprocal -> gpsimd (-mean*rstd) -> scalar apply+cast (Identity
   with bias/scale), each engine loaded under the PE's 853ns/job budget.

Host side: compact/pad/transpose per batch (cheap numpy), run the SPMD NEFF,
scatter results back into the full [Q,B,H] output.
"""

import numpy as np
import ml_dtypes

import concourse.bacc as bacc
import concourse.bass as bass
import concourse.tile as tile
from concourse import mybir
from concourse.bass_utils import run_bass_kernel_spmd


def _ensure_axon_hooks():
    """concourse's trace path imports antenv.axon_hooks, which is absent in
    some containers; provide a no-op stand-in so BASS_TRACE=1 degrades to
    untraced execution instead of crashing."""
    try:
        import antenv.axon_hooks  # noqa: F401
    except ImportError:
        import sys as _sys
        import types as _types
        m = _types.ModuleType("antenv.axon_hooks")
        m._h = None
        m.set_axon_ntff_profile_hook = lambda h: setattr(m, "_h", h)
        m.get_axon_ntff_profile_hook = lambda: m._h
        _sys.modules["antenv.axon_hooks"] = m


_ensure_axon_hooks()

F32 = mybir.dt.float32
BF16 = mybir.dt.bfloat16
AX = mybir.AxisListType.X
AF = mybir.ActivationFunctionType
ALU = mybir.AluOpType

H = 512
HC = H // 128          # contraction chunks over the hidden dim
NCORES = 8
RSQRT_H = 1.0 / float(np.sqrt(np.float32(H)))
EPS = 1e-5
NWARM = 10             # dummy matmuls to lift the HAM clock gate; must bridge
                       # the head DMA wait with NO idle gap, else the HAM
                       # busy-window resets and early jobs run at 1.2 GHz

_cache = {}
last_results = None


def _build(pad, nkfree, biasq, biask, affq, affk):
    nt = pad // 128
    # kT transpose batches double as the QK^T PSUM-bank groups (so each
    # group's matmuls depend on exactly one transpose's tile), sized so each
    # batch's issue+transfer completes before its QK consumers.
    kbat = [min(4, nt)]
    if nt > 4:
        kbat.append(min(3, nt - 4))
    while sum(kbat) < nt:
        kbat.append(min(2, nt - sum(kbat)))
    groups = list(kbat)
    ng = len(groups)
    qbat = [min(3, nt)]
    while sum(qbat) < nt:
        qbat.append(min(3, nt - sum(qbat)))
    # input-tile DMA groups (granularity of arrival -> consumer wakeup):
    # K tiles 0-4 arrive as singles (they gate consecutive early jobs), the
    # tail as one batch; Q tiles in growing batches (need times spread out).
    xqbat = [1] * min(3, nt)
    while sum(xqbat) < nt:
        xqbat.append(min(3, nt - sum(xqbat)))
    xkbat = [1] * min(5, nt)
    while sum(xkbat) < nt:
        xkbat.append(min(2, nt - sum(xkbat)))

    def _bmap(bat):
        m = {}
        base = 0
        for i, b in enumerate(bat):
            for s2 in range(b):
                m[base + s2] = (i, s2, base, b)
            base += b
        return m
    xqmap, xkmap = _bmap(xqbat), _bmap(xkbat)
    qmap, kmap = _bmap(qbat), _bmap(kbat)

    nc = bacc.Bacc(None, target_bir_lowering=False, debug=False, enable_asserts=False,
                   enable_partition_id=False)

    xqT_d = nc.declare_dram_parameter("xqT", [nt, 128, HC, 128], BF16, isOutput=False)
    xkT_d = nc.declare_dram_parameter("xkT", [nt, 128, HC, 128], BF16, isOutput=False)
    v_d = nc.declare_dram_parameter("v", [pad, H], BF16, isOutput=False)
    npad_d = nc.declare_dram_parameter("npad", [1, 1], F32, isOutput=False)
    km_d = None
    if biask or affk:
        km_d = nc.declare_dram_parameter("km01", [pad, 1], F32, isOutput=False)
    wqT_d = nc.declare_dram_parameter("WqT", [H, H], BF16, isOutput=False)
    wkT_d = nc.declare_dram_parameter("WkT", [H, H], BF16, isOutput=False)
    extras_d = {}
    if biasq:
        extras_d["bq"] = nc.declare_dram_parameter("bq", [1, H], F32, isOutput=False)
    if biask:
        extras_d["bk"] = nc.declare_dram_parameter("bk", [1, H], F32, isOutput=False)
    if affq:
        extras_d["gq"] = nc.declare_dram_parameter("gq", [1, H], F32, isOutput=False)
        extras_d["betaq"] = nc.declare_dram_parameter("betaq", [1, H], F32, isOutput=False)
    if affk:
        extras_d["gk"] = nc.declare_dram_parameter("gk", [1, H], F32, isOutput=False)
        extras_d["betak"] = nc.declare_dram_parameter("betak", [1, H], F32, isOutput=False)
    out_d = nc.declare_dram_parameter("out", [pad, H], BF16, isOutput=True)

    with tile.TileContext(nc) as tc:
        with (
            tc.tile_pool(name="persist", bufs=1) as persist,
            tc.tile_pool(name="small", bufs=10) as small,
            tc.tile_pool(name="lnt", bufs=4) as lnt,
            tc.tile_pool(name="ework", bufs=7) as ework,
            tc.tile_pool(name="osb", bufs=3) as osbp,
            tc.tile_pool(name="ps", bufs=1, space="PSUM") as ps,
        ):
            eps_t = persist.tile([128, 1], F32)
            nc.vector.memset(eps_t[:], EPS)
            one_t = persist.tile([128, 1], F32)
            nc.vector.memset(one_t[:], 1.0)
            # warm-up operand: zeros so the dummy matmuls are numerically inert
            wz = persist.tile([128, H], BF16)
            nc.vector.memset(wz[:], 0.0)

            # ---- input DMA choreography --------------------------------------
            # Empirically the SWDGE (gpsimd) queue drains fastest and the
            # scalar HWDGE queue slowest, so: gpsimd carries the weights +
            # first tiles (everything phase A blocks on early), sync carries
            # the bulk x stream then ONLY transposes, scalar (busy computing)
            # carries just xq0 + v + npad.  All issues are unconditional and
            # sit at each engine's stream head.
            wq_h = [persist.tile([128, 2, H], BF16, tag=f"wqh{j}", name=f"wqh{j}")
                    for j in range(2)]
            wk_h = [persist.tile([128, 2, H], BF16, tag=f"wkh{j}", name=f"wkh{j}")
                    for j in range(2)]
            xqb = [persist.tile([128, xqbat[j], HC, 128], BF16, tag=f"xqb{j}", name=f"xqb{j}")
                   for j in range(len(xqbat))]
            xkb = [persist.tile([128, xkbat[j], HC, 128], BF16, tag=f"xkb{j}", name=f"xkb{j}")
                   for j in range(len(xkbat))]

            def _wslice(dram, h):
                return dram[256 * h:256 * (h + 1), :].rearrange("(c p) i -> p c i", p=128)

            def _xslice(dram, bat, j):
                t0 = sum(bat[:j]); t1 = t0 + bat[j]
                return dram[t0:t1, :, :, :].rearrange("t p c u -> p t c u")

            npad_t = persist.tile([128, 1], F32)
            kmA_sb = None
            if km_d is not None:
                kmA_sb = persist.tile([128, nt], F32)
            bcast = {}
            for name in extras_d:
                bcast[name] = persist.tile([128, H], F32, tag=f"bc_{name}", name=f"bc_{name}")

            # [128 h_sub, tile, h_chunk, 128 seq] operand layouts for
            # attention.  kT is one tile PER transpose batch (= PSUM group)
            # so each group's matmuls depend on exactly one DMA; qT likewise.
            qTb = [persist.tile([128, qbat[j], HC, 128], BF16, tag=f"qTb{j}", name=f"qTb{j}")
                   for j in range(len(qbat))]
            kTg = [persist.tile([128, kbat[j], HC, 128], BF16, tag=f"kTg{j}", name=f"kTg{j}")
                   for j in range(len(kbat))]
            lnpK = [persist.tile([128, kbat[j], H], BF16, tag=f"lnpK{j}", name=f"lnpK{j}")
                    for j in range(len(kbat))]
            lnpQ = [persist.tile([128, qbat[j], H], BF16, tag=f"lnpQ{j}", name=f"lnpQ{j}")
                    for j in range(len(qbat))]
            v_sb = persist.tile([128, nt, H], BF16)

            # Each HWDGE/SWDGE ring drains FIFO at ~1/3 of the ~320 GB/s
            # aggregate; pieces are placed so every ring's drain order matches
            # the projection jobs' consumption order (Q0, K0-8, Q1-8), no ring
            # carries more than ~2 pieces ahead of need, and the sync ring is
            # lean so the mid-kernel transposes aren't queued behind bulk.
            def _xq(j):
                return (xqb[j][:, 0:xqbat[j]], _xslice(xqT_d, xqbat, j))

            def _xk(j):
                return (xkb[j][:, 0:xkbat[j]], _xslice(xkT_d, xkbat, j))

            nxk, nxq = len(xkbat), len(xqbat)
            # scalar ring (slowest): only latest-need small pieces + v/npad
            o, i_ = _xq(0); nc.scalar.dma_start(out=o, in_=i_)
            o, i_ = _xk(0); nc.scalar.dma_start(out=o, in_=i_)
            if nxq > 3:
                o, i_ = _xq(3); nc.scalar.dma_start(out=o, in_=i_)
            nc.scalar.dma_start(out=v_sb[:],
                                in_=v_d[:, :].rearrange("(n p) h -> p n h", p=128))
            np_src = npad_d[:, :]
            np_src = bass.AP(tensor=np_src.tensor, offset=np_src.offset,
                             ap=[[0, 128], [1, 1]])
            nc.scalar.dma_start(out=npad_t[:], in_=np_src)
            # gpsimd ring (fastest): wq/wk first halves + odd xk pieces
            nc.gpsimd.dma_start(out=wq_h[0][:], in_=_wslice(wqT_d, 0))
            if nxq > 1:
                o, i_ = _xq(1); nc.gpsimd.dma_start(out=o, in_=i_)
            nc.gpsimd.dma_start(out=wk_h[0][:], in_=_wslice(wkT_d, 0))
            for j in range(1, nxk, 2):
                o, i_ = _xk(j); nc.gpsimd.dma_start(out=o, in_=i_)
            # sync ring: second halves + even xk + tail xq; lean thereafter
            # so the XBAR transposes aren't queued behind bulk
            nc.sync.dma_start(out=wq_h[1][:], in_=_wslice(wqT_d, 1))
            if nxq > 2:
                o, i_ = _xq(2); nc.sync.dma_start(out=o, in_=i_)
            nc.sync.dma_start(out=wk_h[1][:], in_=_wslice(wkT_d, 1))
            for j in range(2, nxk, 2):
                o, i_ = _xk(j); nc.sync.dma_start(out=o, in_=i_)
            for j in range(4, nxq):
                o, i_ = _xq(j); nc.sync.dma_start(out=o, in_=i_)
            if km_d is not None:
                nc.gpsimd.dma_start(out=kmA_sb[:],
                                    in_=km_d[:, :].rearrange("(n p) o -> p (n o)", p=128))
            for name, dram in extras_d.items():
                src = dram[:, :]
                src = bass.AP(tensor=src.tensor, offset=src.offset,
                              ap=[[0, 128]] + [src.ap[-1]])
                nc.gpsimd.dma_start(out=bcast[name][:], in_=src)



            # ---- PE warm-up: lift the HAM clock gate while DMAs stream ------
            for w in range(NWARM):
                pw = ps.tile([128, H], F32, tag="u", bufs=8, name=f"warm{w}")
                nc.tensor.matmul(pw[:], wz[:, 0:128], wz[:], start=True, stop=True)

            # ---- phase A: project + layernorm (XBAR-transposed evacuation) ---
            # First THREE Q jobs: their lnp batch completes at job 2, so the
            # qT0 transpose issues early and the K-side weights aren't needed
            # until job 3 (~2.6us later), easing the head DMA crunch.
            nq0 = min(3, nt)
            jobs = ([(0, t) for t in range(nq0)]
                    + [(1, t) for t in range(nt)]
                    + [(0, t) for t in range(nq0, nt)])
            ps_of = {}
            lnp_of = {}

            def proj(i):
                s, t = jobs[i]
                if s == 0:
                    xi, xs, _, _ = xqmap[t]
                    x_sb = xqb[xi][:, xs, :, :]
                else:
                    xi, xs, _, _ = xkmap[t]
                    x_sb = xkb[xi][:, xs, :, :]
                p = ps.tile([128, H], F32, tag="u", bufs=8)
                ps_of[i] = p
                w_h = (wq_h, wk_h)[s]
                for c in range(HC):
                    nc.tensor.matmul(p[:], x_sb[:, c, :], w_h[c // 2][:, c % 2, :],
                                     start=(c == 0), stop=(c == HC - 1))

            def apply_and_tp(i, p, rstd_ap):
                s, t = jobs[i]
                use_aff = (affq, affk)[s]
                pid, slot, base, bsz = (qmap, kmap)[s][t]
                lnp = (lnpQ, lnpK)[s][pid]
                ln = lnp[:, slot, :]
                # projection outputs are zero-mean by construction (host
                # centers the weights), so LN-apply is a plain scale
                if use_aff:
                    ln32 = lnt.tile([128, H], F32, tag="ln32")
                    nc.scalar.activation(out=ln32[:], in_=p[:], func=AF.Identity,
                                         bias=0.0, scale=rstd_ap)
                    nc.vector.tensor_mul(ln32[:], ln32[:], bcast[("gq", "gk")[s]][:])
                    nc.vector.tensor_add(ln, ln32[:], bcast[("betaq", "betak")[s]][:])
                elif i % 14 == 13:
                    # occasional apply on the vector engine balances scalar
                    # (sqrt+apply ~930/job) against vector (stats+recip ~821)
                    nc.vector.tensor_scalar_mul(ln, p[:], rstd_ap)
                else:
                    nc.scalar.activation(out=ln, in_=p[:], func=AF.Identity,
                                         bias=0.0, scale=rstd_ap)
                if s == 1 and kmA_sb is not None:
                    nc.vector.tensor_scalar_mul(ln, ln, kmA_sb[:, t:t + 1])
                # batched XBAR transpose on sync: per 128-col block b,
                # dst[p, b, c, q] = ln[q, b, c*128+p]
                if slot == bsz - 1:
                    dst = (qTb, kTg)[s][pid][:, 0:bsz, :, :]
                    nc.sync.dma_start_transpose(out=dst, in_=lnp[:, 0:bsz, :])

            def ln_and_tp(i):
                s, t = jobs[i]
                use_bias = (biasq, biask)[s]
                p = ps_of.pop(i)
                if use_bias:
                    nc.vector.tensor_add(p[:], p[:], bcast[("bq", "bk")[s]][:])
                stats = small.tile([128, 6], F32, tag="stats")
                nc.vector.bn_stats(out=stats[:], in_=p[:])
                # zero-mean rows: var*H = nv_even + nv_odd + H*m_even^2; the
                # last term is ~0.4% of var (m_e ~ N(0, 2/H)) -> dropped, the
                # resulting ~0.2% per-row rstd jitter is far under tolerance
                v512 = small.tile([128, 1], F32, tag="v512")
                nc.gpsimd.tensor_tensor(out=v512[:], in0=stats[:, 2:3],
                                        in1=stats[:, 5:6], op=ALU.add)
                sd = small.tile([128, 1], F32, tag="sd")
                nc.scalar.activation(out=sd[:], in_=v512[:], func=AF.Sqrt,
                                     bias=eps_t[:], scale=1.0 / H)
                rstd = small.tile([128, 1], F32, tag="rstd")
                nc.vector.reciprocal(out=rstd[:], in_=sd[:])
                apply_and_tp(i, p, rstd[:])

            DEPTH = 7
            for i in range(min(DEPTH, len(jobs))):
                proj(i)
            for i in range(len(jobs)):
                if i + DEPTH < len(jobs):
                    proj(i + DEPTH)
                ln_and_tp(i)

            # ---- phase B: attention (SW-pipelined) ---------------------------
            eT_sb = persist.tile([128, nt, nt, 128], BF16)
            S_of = {}
            goff = [128 * sum(groups[:j]) for j in range(ng + 1)]

            def qk(t):
                Ss = [ps.tile([128, 128 * groups[j]], F32, tag="u", bufs=8, name=f"S{j}")
                      for j in range(ng)]
                S_of[t] = Ss
                qi, qs, _, _ = qmap[t]
                # group-outer so each score bank finishes (and its exp can
                # start) while the next group's matmuls still stream
                for j in range(ng):
                    for c in range(HC):
                        nc.tensor.matmul(Ss[j][:], qTb[qi][:, qs, c, :],
                                         kTg[j][:, :, c, :],
                                         start=(c == 0), stop=(c == HC - 1))

            def attend(t):
                Ss = S_of.pop(t)
                e = ework.tile([128, pad], BF16, tag="e")
                for j in range(ng):
                    nc.scalar.activation(out=e[:, goff[j]:goff[j + 1]], in_=Ss[j][:],
                                         func=AF.Exp, bias=0.0, scale=RSQRT_H)
                dsum = small.tile([128, 1], F32, tag="dsum")
                nc.vector.reduce_sum(dsum[:], e[:], axis=AX)
                # XBAR transpose: eT[p, kt, q] = e[q, kt*128 + p]
                nc.sync.dma_start_transpose(out=eT_sb[:, t, :, :], in_=e[:])
                # denom = sum(e) - npad  (each padded key contributes e=1;
                # the reference's +0.001 term is ~3.5e-4 of the sum: dropped)
                nc.gpsimd.tensor_scalar(out=dsum[:], in0=dsum[:], scalar1=npad_t[:],
                                        scalar2=None, op0=ALU.subtract)
                r = small.tile([128, 1], F32, tag="r")
                nc.vector.reciprocal(out=r[:], in_=dsum[:])

                if t == nt - 1:
                    # last tile: PV in two half-width chains on separate PSUM
                    # banks so the first half's scale+store overlaps the
                    # second half's matmuls, shortening the kernel tail
                    o = osbp.tile([128, H], BF16, tag="o")
                    for hh in range(2):
                        Ch = ps.tile([128, H // 2], F32, tag="u", bufs=8, name="Ch")
                        for kt in range(nt):
                            nc.tensor.matmul(Ch[:], eT_sb[:, t, kt, :],
                                             v_sb[:, kt, hh * (H // 2):(hh + 1) * (H // 2)],
                                             start=(kt == 0), stop=(kt == nt - 1))
                        oh = o[:, hh * (H // 2):(hh + 1) * (H // 2)]
                        nc.vector.tensor_scalar_mul(oh, Ch[:], r[:])
                        nc.gpsimd.dma_start(
                            out=out_d[t * 128:(t + 1) * 128,
                                      hh * (H // 2):(hh + 1) * (H // 2)],
                            in_=oh)
                    return
                C = ps.tile([128, H], F32, tag="u", bufs=8, name="C")
                for kt in range(nt):
                    nc.tensor.matmul(C[:], eT_sb[:, t, kt, :], v_sb[:, kt, :],
                                     start=(kt == 0), stop=(kt == nt - 1))
                o = osbp.tile([128, H], BF16, tag="o")
                nc.vector.tensor_scalar_mul(o[:], C[:], r[:])
                nc.gpsimd.dma_start(out=out_d[t * 128:(t + 1) * 128, :], in_=o[:])

            BDEPTH = 9
            for t in range(min(BDEPTH, nt)):
                qk(t)
            for t in range(nt):
                if t + BDEPTH < nt:
                    qk(t + BDEPTH)
                attend(t)

    nc.compile()
    return nc


def _get_nc(pad, nkfree, biasq, biask, affq, affk):
    key = (pad, nkfree, biasq, biask, affq, affk)
    if key not in _cache:
        _cache[key] = _build(*key)
    return _cache[key]


def kernel(query, key_in, value, query_mask, key_mask,
           Wq, bq, gq, betaq, Wk, bk, gk, betak):
    query = np.asarray(query, np.float32)
    key_in = np.asarray(key_in, np.float32)
    value = np.asarray(value, np.float32)
    query_mask = np.asarray(query_mask, bool)
    key_mask = np.asarray(key_mask, bool)
    Wq = np.asarray(Wq, np.float32); Wk = np.asarray(Wk, np.float32)
    bq = np.asarray(bq, np.float32); bk = np.asarray(bk, np.float32)
    gq = np.asarray(gq, np.float32); gk = np.asarray(gk, np.float32)
    betaq = np.asarray(betaq, np.float32); betak = np.asarray(betak, np.float32)

    Q, B, Hh = query.shape
    assert Hh == H and B == NCORES

    qidx = [np.nonzero(query_mask[:, b])[0] for b in range(B)]
    kidx = [np.nonzero(key_mask[:, b])[0] for b in range(B)]
    maxn = max([len(i) for i in qidx + kidx] + [1])
    pad = max(1152, -(-maxn // 128) * 128)

    biasq = bool(np.any(bq)); biask = bool(np.any(bk))
    affq = not (np.all(gq == 1.0) and not np.any(betaq))
    affk = not (np.all(gk == 1.0) and not np.any(betak))
    # number of key columns actually scored (max real keys across cores)
    nkfree = min(pad, max(max(len(i) for i in kidx), 1))
    nc = _get_nc(pad, nkfree, biasq, biask, affq, affk)

    # center the projections on the host: y = x @ (W - mean_rows(W))^T (+ b -
    # mean(b)) has exactly zero row-mean, so the kernel's LN skips the mean
    # entirely (its bn_stats variance formula relies on this)
    Wqc = Wq - Wq.mean(axis=0, keepdims=True)
    Wkc = Wk - Wk.mean(axis=0, keepdims=True)
    bq = bq - bq.mean(); bk = bk - bk.mean()
    wqT = np.ascontiguousarray(Wqc.T).astype(ml_dtypes.bfloat16)
    wkT = np.ascontiguousarray(Wkc.T).astype(ml_dtypes.bfloat16)
    in_maps = []
    for b in range(B):
        qi, ki = qidx[b], kidx[b]
        xq = np.zeros((pad, H), ml_dtypes.bfloat16)
        xq[:len(qi)] = query[qi, b].astype(ml_dtypes.bfloat16)
        xk = np.zeros((pad, H), ml_dtypes.bfloat16)
        xk[:len(ki)] = key_in[ki, b].astype(ml_dtypes.bfloat16)
        vv = np.zeros((pad, H), ml_dtypes.bfloat16)
        vv[:len(ki)] = value[ki, b].astype(ml_dtypes.bfloat16)
        nt = pad // 128
        # tile-major layout [nt, 128(p), HC, 128(u)]: per-tile DMA reads are
        # 1KB-contiguous per partition
        xqt = np.ascontiguousarray(xq.reshape(nt, 128, H // 128, 128).transpose(0, 3, 2, 1))
        xkt = np.ascontiguousarray(xk.reshape(nt, 128, H // 128, 128).transpose(0, 3, 2, 1))
        m = {
            "xqT": xqt,
            "xkT": xkt,
            "v": vv,
            "npad": np.full((1, 1), pad - len(ki), np.float32),
            "WqT": wqT,
            "WkT": wkT,
        }
        if biask or affk:
            km01 = np.zeros((pad, 1), np.float32); km01[:len(ki)] = 1.0
            m["km01"] = km01
        if biasq: m["bq"] = bq.reshape(1, H)
        if biask: m["bk"] = bk.reshape(1, H)
        if affq: m["gq"] = gq.reshape(1, H); m["betaq"] = betaq.reshape(1, H)
        if affk: m["gk"] = gk.reshape(1, H); m["betak"] = betak.reshape(1, H)
        in_maps.append(m)

    res = run_bass_kernel_spmd(nc, in_maps, core_ids=list(range(NCORES)))
    global last_results
    last_results = res

    out = np.zeros((Q, B, H), np.float32)
    for b in range(B):
        qi = qidx[b]
        out[qi, b, :] = res.results[b]["out"][:len(qi)]
    return out
